# revision 23
# baseline (speedup 1.0000x reference)
"""AttentionBlock kernel for 8 Trainium2 NeuronCores.

Reference computation (per batch element b of 8):
    xn  = GroupNorm(x, 32 groups, eps=1e-5) * gn_scale + gn_bias
    qkv = w_qkv @ xn + b_qkv          (1x1 conv == channel matmul)
    q, k, v = split(qkv)              each (C=256, N=4096)
    S   = (q^T k) * C^-0.5            (N, N) scores
    A   = softmax(S, axis=-1)
    AO  = (A @ v^T)^T                 (C, N)
    out = w_out @ AO + b_out + x

Sharding: data-parallel over batch — core i computes batch element i.

Per-core layout strategy (everything channel-chunked into 2x128 partitions):
  - GroupNorm stats per channel via bn_stats/bn_aggr (free-dim reduce), then
    cross-partition group aggregation via tiny selector matmuls on the PE.
  - q, k stored (c, n); scores computed *transposed*: S^T tile (m=128, nb=512)
    = k_slice^T @ q_block, so exp(S^T) tiles are directly the lhsT for the
    second matmul. Softmax skips max-subtraction (scores are ~N(0,1); |s| < 10
    for randn inputs, exp is safely in fp32/bf16 range; identical math to
    softmax-with-max).
  - v is produced directly transposed (vT: m-part, c-free) by the projection,
    with a ones-column appended, so AO^T = exp(S^T)^T @ [vT|1] yields both the
    unnormalized attention output AND the softmax denominator in one pass.
  - AO^T rows are normalized (per-partition scalar), transposed back 128x128
    on the PE, projected with w_out, bias+residual added on eviction.
Matmul inputs are bf16 (fp32 PSUM accumulation); rel err vs fp32 ref ~1e-3.
"""

import numpy as np

import concourse.bass as bass
import concourse.bacc as bacc
import concourse.mybir as mybir
import concourse.tile as tile
from concourse.bass_utils import run_bass_kernel_spmd
from concourse.masks import make_identity

F32 = mybir.dt.float32
BF16 = mybir.dt.bfloat16
FP8 = mybir.dt.float8e4
ESC_BIAS = -3.4657359027997265  # ln(1/32): exp scaled into fp8e4m3 range
VPAD = 16                       # vt free-dim pad so the DR middle step %16==0

B = 8          # batch / cores
C = 256        # channels
P = 128        # partitions
CK = C // P    # channel chunks (2)
H = W = 64
N = H * W      # 4096 spatial positions
NB = 512       # query-block width (free dim)
NBLK = N // NB  # 8 query blocks
MT = N // P    # 32 key tiles of 128
G = 32         # groups
GS = C // G    # channels per group (8)
EPS = 1e-5
SCALE = float(C) ** -0.5
O_QKV = 3 * C  # 768
BN_SUB = 512   # bn_stats subgroup width



_TILE_FREES = []


def _tile(tc, *args, **kwargs):
    t, free = tc.tile(*args, **kwargs)
    _TILE_FREES.append(free)  # keep persistent tiles alive (GC would release)
    return t

def build_attention_program(nc, n=N):
    """Emit the full single-core program into `nc` (one batch element)."""
    nblk = max(1, n // NB)       # query blocks
    mtn = n // P                 # key tiles
    nbsub = min(n, BN_SUB)       # bn_stats subgroup width
    GL = P // GS                 # groups per channel chunk (16)
    x_d = nc.dram_tensor("x", [C, n], F32, kind="ExternalInput").ap()
    gns_d = nc.dram_tensor("gn_scale", [C], F32, kind="ExternalInput").ap()
    gnb_d = nc.dram_tensor("gn_bias", [C], F32, kind="ExternalInput").ap()
    wqkv_d = nc.dram_tensor("w_qkv", [O_QKV, C], F32, kind="ExternalInput").ap()
    bqkv_d = nc.dram_tensor("b_qkv", [O_QKV], F32, kind="ExternalInput").ap()
    wout_d = nc.dram_tensor("w_out", [C, C], F32, kind="ExternalInput").ap()
    bout_d = nc.dram_tensor("b_out", [C], F32, kind="ExternalInput").ap()
    out_d = nc.dram_tensor("out", [C, n], F32, kind="ExternalOutput").ap()

    from contextlib import ExitStack

    with tile.TileContext(nc) as tc, ExitStack() as ctx:
        # ---------------- persistent SBUF ----------------
        x_sb = [_tile(tc, [P, n], F32, name=f"x_sb{j}") for j in range(CK)]
        xn_sb = [_tile(tc, [P, n], BF16, name=f"xn_sb{j}") for j in range(CK)]
        q3 = _tile(tc, [P, CK, n], FP8, name="q3")
        k3 = _tile(tc, [P, CK, n], FP8, name="k3")
        vt_sb = _tile(tc, [P, mtn, C + VPAD], FP8, name="vt_sb")
        expT = _tile(tc, [P, mtn, NB], FP8, name="expT")
        wqkvT = [_tile(tc, [P, O_QKV], BF16, name=f"wqkvT{j}") for j in range(CK)]
        woutT = [_tile(tc, [P, C], BF16, name=f"woutT{j}") for j in range(CK)]
        ident_f = _tile(tc, [P, P], F32, name="ident_f")
        ident_b = _tile(tc, [P, P], BF16, name="ident_b")

        # small per-channel vectors
        scale_sb = [_tile(tc, [P, 1], F32, name=f"scale_sb{j}") for j in range(CK)]
        bias_sb = [_tile(tc, [P, 1], F32, name=f"bias_sb{j}") for j in range(CK)]
        bq_sb = [_tile(tc, [P, 1], F32, name=f"bq_sb{j}") for j in range(CK)]
        bk_sb = [_tile(tc, [P, 1], F32, name=f"bk_sb{j}") for j in range(CK)]
        bo_sb = [_tile(tc, [P, 1], F32, name=f"bo_sb{j}") for j in range(CK)]
        bv_bc = _tile(tc, [P, C], F32, name="bv_bc")
        sel = [_tile(tc, [P, GL], F32, name=f"sel{j}") for j in range(CK)]
        selT = [_tile(tc, [GL, P], F32, name=f"selT{j}") for j in range(CK)]
        eps_sb = _tile(tc, [GL, 1], F32, name="eps_sb")
        eln_sb = _tile(tc, [P, 1], F32, name="eln_sb")

        # ---------------- pools (after singles: LIFO release order) -----
        ps_s = ctx.enter_context(tc.tile_pool(name="ps_s", bufs=2, space="PSUM"))
        ps_a = ctx.enter_context(tc.tile_pool(name="ps_a", bufs=4, space="PSUM"))
        work = ctx.enter_context(tc.tile_pool(name="work", bufs=3))
        evac = ctx.enter_context(tc.tile_pool(name="evac", bufs=3))

        # ---------------- input DMA ----------------
        npieces = max(1, n // BN_SUB)
        pw = n // npieces
        for j in range(CK):
            for piece in range(npieces):
                nc.sync.dma_start(
                    out=x_sb[j][:, piece * pw:(piece + 1) * pw],
                    in_=x_d[j * P:(j + 1) * P, piece * pw:(piece + 1) * pw],
                )
        wq_raw = []
        for i in range(O_QKV // P):
            t = work.tile([P, C], F32, tag="wraw", name=f"wqraw{i}")
            nc.sync.dma_start(out=t, in_=wqkv_d[i * P:(i + 1) * P, :])
            wq_raw.append(t)
        wo_raw = []
        for i in range(CK):
            t = work.tile([P, C], F32, tag="wraw2", name=f"woraw{i}")
            nc.sync.dma_start(out=t, in_=wout_d[i * P:(i + 1) * P, :])
            wo_raw.append(t)
        for j in range(CK):
            sl = slice(j * P, (j + 1) * P)
            nc.sync.dma_start(out=scale_sb[j], in_=gns_d[sl].rearrange("(a u) -> a u", u=1))
            nc.sync.dma_start(out=bias_sb[j], in_=gnb_d[sl].rearrange("(a u) -> a u", u=1))
            nc.sync.dma_start(out=bq_sb[j], in_=bqkv_d[sl].rearrange("(a u) -> a u", u=1))
            nc.sync.dma_start(
                out=bk_sb[j],
                in_=bqkv_d[C + j * P:C + (j + 1) * P].rearrange("(a u) -> a u", u=1),
            )
            nc.sync.dma_start(out=bo_sb[j], in_=bout_d[sl].rearrange("(a u) -> a u", u=1))
        bv_src = bqkv_d[2 * C:3 * C]
        nc.sync.dma_start(
            out=bv_bc,
            in_=bass.AP(tensor=bv_src.tensor, offset=bv_src.offset,
                        ap=[[0, P]] + list(bv_src.ap)),
        )

        # ---------------- constants ----------------
        make_identity(nc, ident_f)
        make_identity(nc, ident_b)
        nc.vector.memset(eps_sb, EPS)
        nc.vector.memset(eln_sb, ESC_BIAS)
        # per-chunk local selectors: sel[c, g] = 1/GS where c//GS == g
        for j in range(CK):
            nc.gpsimd.memset(sel[j], 0.0)
            nc.gpsimd.affine_select(
                out=sel[j], in_=sel[j], compare_op=mybir.AluOpType.is_gt,
                fill=1.0 / GS, base=1 - GS, pattern=[[-GS, GL]],
                channel_multiplier=1,
            )
            nc.gpsimd.affine_select(
                out=sel[j], in_=sel[j], compare_op=mybir.AluOpType.is_ge,
                fill=0.0, base=0, pattern=[[-GS, GL]], channel_multiplier=1,
            )
            nc.gpsimd.memset(selT[j], 0.0)
            nc.gpsimd.affine_select(
                out=selT[j], in_=selT[j], compare_op=mybir.AluOpType.is_gt,
                fill=1.0, base=1 - GS, pattern=[[1, P]], channel_multiplier=-GS,
            )
            nc.gpsimd.affine_select(
                out=selT[j], in_=selT[j], compare_op=mybir.AluOpType.is_ge,
                fill=0.0, base=0, pattern=[[1, P]], channel_multiplier=-GS,
            )

        # q/k bias folded with the attention scale: q' = (q + bq) * SCALE
        bqs_sb = []
        for j in range(CK):
            t = work.tile([P, 1], F32, tag="bqs", name=f"bqs{j}")
            nc.vector.tensor_scalar_mul(out=t, in0=bq_sb[j], scalar1=SCALE)
            bqs_sb.append(t)

        # ---------------- weight transposes (PE) ----------------
        for i in range(O_QKV // P):
            for j in range(CK):
                pt = ps_a.tile([P, P], F32, tag="a", name="wtp")
                nc.tensor.transpose(pt, wq_raw[i][:, j * P:(j + 1) * P], ident_f)
                nc.vector.tensor_copy(wqkvT[j][:, i * P:(i + 1) * P], pt)
        for i in range(CK):
            for j in range(CK):
                pt = ps_a.tile([P, P], F32, tag="a", name="wtp2")
                nc.tensor.transpose(pt, wo_raw[i][:, j * P:(j + 1) * P], ident_f)
                nc.vector.tensor_copy(woutT[j][:, i * P:(i + 1) * P], pt)

        # ---------------- group norm (per-chunk independent) -----------
        for j in range(CK):
            stats = work.tile([P, n // nbsub, 6], F32, tag="bnst", name=f"bnst{j}")
            xr = x_sb[j][:].rearrange("p (s d) -> p s d", d=nbsub)
            for s in range(n // nbsub):
                nc.vector.bn_stats(out=stats[:, s, :], in_=xr[:, s, :])
            mv = work.tile([P, 2], F32, tag="mv", name=f"mv{j}")
            nc.vector.bn_aggr(out=mv, in_=stats)
            # mv2 = [mean, E[x^2]] per channel
            m2 = work.tile([P, 1], F32, tag="m2", name=f"m2{j}")
            nc.vector.tensor_mul(m2, mv[:, 0:1], mv[:, 0:1])
            mv2 = work.tile([P, 2], F32, tag="mv2", name=f"mv2{j}")
            nc.vector.tensor_copy(mv2[:, 0:1], mv[:, 0:1])
            nc.vector.tensor_add(mv2[:, 1:2], mv[:, 1:2], m2)
            ps_g = ps_a.tile([GL, 2], F32, tag="a", name="ps_g")
            nc.tensor.matmul(ps_g, sel[j], mv2, start=True, stop=True)
            gs = work.tile([GL, 2], F32, tag="gs", name=f"gs{j}")
            nc.vector.tensor_copy(gs, ps_g)
            gm2 = work.tile([GL, 1], F32, tag="gm2", name=f"gm2{j}")
            nc.vector.tensor_mul(gm2, gs[:, 0:1], gs[:, 0:1])
            gvar = work.tile([GL, 1], F32, tag="gvar", name=f"gvar{j}")
            nc.vector.tensor_sub(gvar, gs[:, 1:2], gm2)
            gsd = work.tile([GL, 1], F32, tag="gsd", name=f"gsd{j}")
            nc.scalar.activation(out=gsd, in_=gvar,
                                 func=mybir.ActivationFunctionType.Sqrt,
                                 bias=eps_sb, scale=1.0)
            grstd = work.tile([GL, 1], F32, tag="grstd", name=f"grstd{j}")
            nc.vector.reciprocal(grstd, gsd)
            gstat2 = work.tile([GL, 2], F32, tag="gstat2", name=f"gstat2{j}")
            nc.vector.tensor_copy(gstat2[:, 0:1], gs[:, 0:1])
            nc.vector.tensor_copy(gstat2[:, 1:2], grstd)
            ps_bc = ps_a.tile([P, 2], F32, tag="a", name="ps_bc")
            nc.tensor.matmul(ps_bc, selT[j], gstat2, start=True, stop=True)
            a_c = work.tile([P, 1], F32, tag="a_c", name=f"a_c{j}")
            nc.vector.tensor_mul(a_c, ps_bc[:, 1:2], scale_sb[j])
            t_c = work.tile([P, 1], F32, tag="t_c", name=f"t_c{j}")
            nc.vector.tensor_mul(t_c, ps_bc[:, 0:1], a_c)
            b_c = work.tile([P, 1], F32, tag="b_c", name=f"b_c{j}")
            nc.vector.tensor_sub(b_c, bias_sb[j], t_c)
            if j == 0:
                nc.vector.tensor_scalar(
                    out=xn_sb[j], in0=x_sb[j], scalar1=a_c, scalar2=b_c,
                    op0=mybir.AluOpType.mult, op1=mybir.AluOpType.add,
                )
            else:
                # xn = Identity(x * A + B) on ACT frees the DVE
                nc.scalar.activation(
                    out=xn_sb[j], in_=x_sb[j],
                    func=mybir.ActivationFunctionType.Identity,
                    bias=b_c, scale=a_c,
                )

        # preload the exp table set so the first real exp doesn't stall
        # ~1.5us on ACT_TABLE_LOAD (sqrt set was loaded during GroupNorm)
        dummy_exp = work.tile([1, 1], F32, tag="dummy", name="dummy_exp")
        nc.scalar.activation(out=dummy_exp, in_=eps_sb[0:1, :],
                             func=mybir.ActivationFunctionType.Exp)

        # ---------------- q, k projections ----------------
        for nb in range(nblk):
            for idx, (dst, bias_ap, sc) in enumerate(
                [(q3, bqs_sb, SCALE), (k3, bk_sb, 1.0)]
            ):
                for oc in range(CK):
                    o_off = idx * C + oc * P
                    ps = ps_a.tile([P, NB], F32, tag="a", name="ps_qk")
                    for kc in range(CK):
                        nc.tensor.matmul(
                            ps,
                            wqkvT[kc][:, o_off:o_off + P],
                            xn_sb[kc][:, nb * NB:(nb + 1) * NB],
                            start=(kc == 0), stop=(kc == CK - 1),
                        )
                    if (nb + idx) % 2 == 0:
                        nc.vector.tensor_scalar(
                            out=dst[:, oc, nb * NB:(nb + 1) * NB], in0=ps,
                            scalar1=sc, scalar2=bias_ap[oc],
                            op0=mybir.AluOpType.mult, op1=mybir.AluOpType.add,
                        )
                    else:
                        nc.scalar.activation(
                            out=dst[:, oc, nb * NB:(nb + 1) * NB], in_=ps,
                            func=mybir.ActivationFunctionType.Identity,
                            bias=bias_ap[oc], scale=sc,
                        )

        # ---------------- vT projection (m-part, c-free, +ones col) ------
        nc.gpsimd.memset(vt_sb[:, :, C:C + VPAD], 0.0)
        nc.gpsimd.memset(vt_sb[:, :, C:C + 1], 1.0)
        for mt in range(mtn):
            ps = ps_a.tile([P, NB], F32, tag="a", name="ps_v")
            for kc in range(CK):
                nc.tensor.matmul(
                    ps[:, 0:C],
                    xn_sb[kc][:, mt * P:(mt + 1) * P],
                    wqkvT[kc][:, 2 * C:3 * C],
                    start=(kc == 0), stop=(kc == CK - 1),
                )
            nc.vector.tensor_add(vt_sb[:, mt, 0:C], ps[:, 0:C], bv_bc)

        # ---------------- attention blocks ----------------
        # fp8 DoubleRow: each S matmul contracts K=256 (both channel chunks
        # paired per PE cell); each AO matmul contracts a PAIR of key tiles.
        # The 4 AO accumulation chains trail the exp front inside the
        # S-phase. Each block's normalize result is held and its
        # transpose/project/store tail is emitted early in the NEXT block's
        # S-phase — after the previous chains' psum slots are freed but
        # BEFORE this block's chains claim theirs — so the PE never parks
        # at a block boundary while ACT still has exp backlog.
        expT3 = expT[:]
        DR = mybir.MatmulPerfMode.DoubleRow
        LAG = 4
        npairs = mtn // 2
        nsubs = NB // P

        def emit_s_pair(blk, p):
            nsl = slice(blk * NB, (blk + 1) * NB)
            ps = ps_s.tile([P, 2, NB], F32, tag="s", name="ps_s")
            for sub in range(2):
                mt = 2 * p + sub
                nc.tensor.matmul(
                    ps[:, sub, :],
                    k3[:, :, mt * P:(mt + 1) * P],
                    q3[:, :, nsl],
                    perf_mode=DR, start=True, stop=True,
                )
            nc.scalar.activation(
                out=expT3[:, 2 * p:2 * p + 2, :], in_=ps,
                func=mybir.ActivationFunctionType.Exp,
                bias=eln_sb,
            )

        def emit_ao_pair(ao_ps, j):
            for c in range(nsubs):
                nc.tensor.matmul(
                    ao_ps[c][:, 0:C + VPAD],
                    expT3[:, 2 * j:2 * j + 2, c * P:(c + 1) * P],
                    vt_sb[:, 2 * j:2 * j + 2, :],
                    perf_mode=DR,
                    start=(j == 0), stop=(j == npairs - 1),
                )

        def emit_tail(blk, aots):
            nsl = slice(blk * NB, (blk + 1) * NB)
            ao_sb = [
                evac.tile([P, NB], BF16, tag=f"ao_sb{j}", name=f"ao_sb{j}")
                for j in range(CK)
            ]
            for c in range(nsubs):
                for j in range(CK):
                    pt = ps_a.tile([P, P], BF16, tag="a", name="ao_tp")
                    nc.tensor.transpose(pt, aots[c][:, j * P:(j + 1) * P],
                                        ident_b)
                    nc.vector.tensor_copy(ao_sb[j][:, c * P:(c + 1) * P], pt)
            for oc in range(CK):
                ps = ps_a.tile([P, NB], F32, tag="a", name="ps_f")
                for kc in range(CK):
                    nc.tensor.matmul(
                        ps,
                        woutT[kc][:, oc * P:(oc + 1) * P],
                        ao_sb[kc],
                        start=(kc == 0), stop=(kc == CK - 1),
                    )
                f_sb = evac.tile([P, NB], F32, tag="f_sb", name="f_sb")
                nc.scalar.activation(out=f_sb, in_=ps,
                                     func=mybir.ActivationFunctionType.Identity,
                                     bias=bo_sb[oc], scale=1.0)
                o_sb = evac.tile([P, NB], F32, tag="o_sb", name="o_sb")
                nc.vector.tensor_add(o_sb, f_sb, x_sb[oc][:, nsl])
                nc.sync.dma_start(out=out_d[oc * P:(oc + 1) * P, nsl], in_=o_sb)

        pending = None
        for blk in range(nblk):
            lag = min(LAG, npairs - 1)
            ao_ps = None
            for p in range(npairs):
                emit_s_pair(blk, p)
                if p == min(2, lag - 1) and pending is not None:
                    emit_tail(*pending)
                    pending = None
                if p == lag:
                    ao_ps = [ps_a.tile([P, NB], F32, tag="a", name=f"ps_ao{c}")
                             for c in range(nsubs)]
                if p >= lag:
                    emit_ao_pair(ao_ps, p - lag)
            for j in range(npairs - lag, npairs):
                emit_ao_pair(ao_ps, j)
            aots = []
            for c in range(nsubs):
                ps = ao_ps[c]
                recip = work.tile([P, 1], F32, tag="recip", name="recip")
                nc.vector.reciprocal(recip, ps[:, C:C + 1])
                aot = evac.tile([P, C], BF16, tag="aot", bufs=8, name="aot")
                nc.vector.tensor_scalar_mul(out=aot, in0=ps[:, 0:C], scalar1=recip)
                aots.append(aot)
            pending = (blk, aots)
        emit_tail(*pending)

    return nc


_CACHED_NC = {}


def build_nc(n=N):
    if n not in _CACHED_NC:
        nc = bacc.Bacc("TRN2", target_bir_lowering=False, debug=False,
                       num_devices=B)
        build_attention_program(nc, n=n)
        nc.compile()
        _CACHED_NC[n] = nc
    return _CACHED_NC[n]


def make_in_maps(x, gn_scale, gn_bias, w_qkv, b_qkv, w_out, b_out):
    f = np.ascontiguousarray
    return [
        {
            "x": f(x[b].reshape(C, N), dtype=np.float32),
            "gn_scale": f(gn_scale, dtype=np.float32),
            "gn_bias": f(gn_bias, dtype=np.float32),
            "w_qkv": f(w_qkv, dtype=np.float32),
            "b_qkv": f(b_qkv, dtype=np.float32),
            "w_out": f(w_out, dtype=np.float32),
            "b_out": f(b_out, dtype=np.float32),
        }
        for b in range(B)
    ]


def kernel(x, gn_scale, gn_bias, w_qkv, b_qkv, w_out, b_out, _trace=False,
           _tmpdir=None):
    x = np.asarray(x)
    nc = build_nc()
    in_maps = make_in_maps(x, gn_scale, gn_bias, w_qkv, b_qkv, w_out, b_out)
    res = run_bass_kernel_spmd(nc, in_maps, list(range(B)), trace=_trace,
                               tmpdir=_tmpdir)
    out = np.stack([res.results[b]["out"] for b in range(B)])
    out = out.reshape(B, C, H, W).astype(np.float32)
    if _trace:
        kernel.last_exec_time_ns = res.exec_time_ns
        kernel.last_results = res
    return out


# revision 24
# speedup vs baseline: 1.0085x; 1.0085x over previous
"""AttentionBlock kernel for 8 Trainium2 NeuronCores.

Reference computation (per batch element b of 8):
    xn  = GroupNorm(x, 32 groups, eps=1e-5) * gn_scale + gn_bias
    qkv = w_qkv @ xn + b_qkv          (1x1 conv == channel matmul)
    q, k, v = split(qkv)              each (C=256, N=4096)
    S   = (q^T k) * C^-0.5            (N, N) scores
    A   = softmax(S, axis=-1)
    AO  = (A @ v^T)^T                 (C, N)
    out = w_out @ AO + b_out + x

Sharding: data-parallel over batch — core i computes batch element i.

Per-core layout strategy (everything channel-chunked into 2x128 partitions):
  - GroupNorm stats per channel via bn_stats/bn_aggr (free-dim reduce), then
    cross-partition group aggregation via tiny selector matmuls on the PE.
  - q, k stored (c, n); scores computed *transposed*: S^T tile (m=128, nb=512)
    = k_slice^T @ q_block, so exp(S^T) tiles are directly the lhsT for the
    second matmul. Softmax skips max-subtraction (scores are ~N(0,1); |s| < 10
    for randn inputs, exp is safely in fp32/bf16 range; identical math to
    softmax-with-max).
  - v is produced directly transposed (vT: m-part, c-free) by the projection,
    with a ones-column appended, so AO^T = exp(S^T)^T @ [vT|1] yields both the
    unnormalized attention output AND the softmax denominator in one pass.
  - AO^T rows are normalized (per-partition scalar), transposed back 128x128
    on the PE, projected with w_out, bias+residual added on eviction.
Matmul inputs are bf16 (fp32 PSUM accumulation); rel err vs fp32 ref ~1e-3.
"""

import numpy as np

import concourse.bass as bass
import concourse.bacc as bacc
import concourse.mybir as mybir
import concourse.tile as tile
from concourse.bass_utils import run_bass_kernel_spmd
from concourse.masks import make_identity

F32 = mybir.dt.float32
BF16 = mybir.dt.bfloat16
FP8 = mybir.dt.float8e4
ESC_BIAS = -3.4657359027997265  # ln(1/32): exp scaled into fp8e4m3 range
VPAD = 16                       # vt free-dim pad so the DR middle step %16==0

B = 8          # batch / cores
C = 256        # channels
P = 128        # partitions
CK = C // P    # channel chunks (2)
H = W = 64
N = H * W      # 4096 spatial positions
NB = 512       # query-block width (free dim)
NBLK = N // NB  # 8 query blocks
MT = N // P    # 32 key tiles of 128
G = 32         # groups
GS = C // G    # channels per group (8)
EPS = 1e-5
SCALE = float(C) ** -0.5
O_QKV = 3 * C  # 768
BN_SUB = 512   # bn_stats subgroup width



_TILE_FREES = []


def _tile(tc, *args, **kwargs):
    t, free = tc.tile(*args, **kwargs)
    _TILE_FREES.append(free)  # keep persistent tiles alive (GC would release)
    return t

def build_attention_program(nc, n=N):
    """Emit the full single-core program into `nc` (one batch element)."""
    nblk = max(1, n // NB)       # query blocks
    mtn = n // P                 # key tiles
    nbsub = min(n, BN_SUB)       # bn_stats subgroup width
    GL = P // GS                 # groups per channel chunk (16)
    x_d = nc.dram_tensor("x", [C, n], F32, kind="ExternalInput").ap()
    gns_d = nc.dram_tensor("gn_scale", [C], F32, kind="ExternalInput").ap()
    gnb_d = nc.dram_tensor("gn_bias", [C], F32, kind="ExternalInput").ap()
    wqkv_d = nc.dram_tensor("w_qkv", [O_QKV, C], F32, kind="ExternalInput").ap()
    bqkv_d = nc.dram_tensor("b_qkv", [O_QKV], F32, kind="ExternalInput").ap()
    wout_d = nc.dram_tensor("w_out", [C, C], F32, kind="ExternalInput").ap()
    bout_d = nc.dram_tensor("b_out", [C], F32, kind="ExternalInput").ap()
    out_d = nc.dram_tensor("out", [C, n], F32, kind="ExternalOutput").ap()

    from contextlib import ExitStack

    with tile.TileContext(nc) as tc, ExitStack() as ctx:
        # ---------------- persistent SBUF ----------------
        x_sb = [_tile(tc, [P, n], F32, name=f"x_sb{j}") for j in range(CK)]
        xn_sb = [_tile(tc, [P, n], BF16, name=f"xn_sb{j}") for j in range(CK)]
        q3 = _tile(tc, [P, CK, n], FP8, name="q3")
        k3 = _tile(tc, [P, CK, n], FP8, name="k3")
        vt_sb = _tile(tc, [P, mtn, C + VPAD], FP8, name="vt_sb")
        expT = _tile(tc, [P, mtn, NB], FP8, name="expT")
        wqkvT = [_tile(tc, [P, O_QKV], BF16, name=f"wqkvT{j}") for j in range(CK)]
        woutT = [_tile(tc, [P, C], BF16, name=f"woutT{j}") for j in range(CK)]
        ident_f = _tile(tc, [P, P], F32, name="ident_f")
        ident_b = _tile(tc, [P, P], BF16, name="ident_b")

        # small per-channel vectors
        scale_sb = [_tile(tc, [P, 1], F32, name=f"scale_sb{j}") for j in range(CK)]
        bias_sb = [_tile(tc, [P, 1], F32, name=f"bias_sb{j}") for j in range(CK)]
        bq_sb = [_tile(tc, [P, 1], F32, name=f"bq_sb{j}") for j in range(CK)]
        bk_sb = [_tile(tc, [P, 1], F32, name=f"bk_sb{j}") for j in range(CK)]
        bo_sb = [_tile(tc, [P, 1], F32, name=f"bo_sb{j}") for j in range(CK)]
        bv_bc = _tile(tc, [P, C], F32, name="bv_bc")
        sel = [_tile(tc, [P, GL], F32, name=f"sel{j}") for j in range(CK)]
        selT = [_tile(tc, [GL, P], F32, name=f"selT{j}") for j in range(CK)]
        eps_sb = _tile(tc, [GL, 1], F32, name="eps_sb")
        eln_sb = _tile(tc, [P, 1], F32, name="eln_sb")

        # ---------------- pools (after singles: LIFO release order) -----
        ps_s = ctx.enter_context(tc.tile_pool(name="ps_s", bufs=2, space="PSUM"))
        ps_a = ctx.enter_context(tc.tile_pool(name="ps_a", bufs=4, space="PSUM"))
        work = ctx.enter_context(tc.tile_pool(name="work", bufs=3))
        evac = ctx.enter_context(tc.tile_pool(name="evac", bufs=3))

        # ---------------- input DMA ----------------
        npieces = max(1, n // BN_SUB)
        pw = n // npieces
        for j in range(CK):
            for piece in range(npieces):
                nc.sync.dma_start(
                    out=x_sb[j][:, piece * pw:(piece + 1) * pw],
                    in_=x_d[j * P:(j + 1) * P, piece * pw:(piece + 1) * pw],
                )
        wq_raw = []
        for i in range(O_QKV // P):
            t = work.tile([P, C], F32, tag="wraw", name=f"wqraw{i}")
            nc.sync.dma_start(out=t, in_=wqkv_d[i * P:(i + 1) * P, :])
            wq_raw.append(t)
        wo_raw = []
        for i in range(CK):
            t = work.tile([P, C], F32, tag="wraw2", name=f"woraw{i}")
            nc.sync.dma_start(out=t, in_=wout_d[i * P:(i + 1) * P, :])
            wo_raw.append(t)
        for j in range(CK):
            sl = slice(j * P, (j + 1) * P)
            nc.sync.dma_start(out=scale_sb[j], in_=gns_d[sl].rearrange("(a u) -> a u", u=1))
            nc.sync.dma_start(out=bias_sb[j], in_=gnb_d[sl].rearrange("(a u) -> a u", u=1))
            nc.sync.dma_start(out=bq_sb[j], in_=bqkv_d[sl].rearrange("(a u) -> a u", u=1))
            nc.sync.dma_start(
                out=bk_sb[j],
                in_=bqkv_d[C + j * P:C + (j + 1) * P].rearrange("(a u) -> a u", u=1),
            )
            nc.sync.dma_start(out=bo_sb[j], in_=bout_d[sl].rearrange("(a u) -> a u", u=1))
        bv_src = bqkv_d[2 * C:3 * C]
        nc.sync.dma_start(
            out=bv_bc,
            in_=bass.AP(tensor=bv_src.tensor, offset=bv_src.offset,
                        ap=[[0, P]] + list(bv_src.ap)),
        )

        # ---------------- constants ----------------
        make_identity(nc, ident_f)
        make_identity(nc, ident_b)
        nc.vector.memset(eps_sb, EPS)
        nc.vector.memset(eln_sb, ESC_BIAS)
        # per-chunk local selectors: sel[c, g] = 1/GS where c//GS == g
        for j in range(CK):
            nc.gpsimd.memset(sel[j], 0.0)
            nc.gpsimd.affine_select(
                out=sel[j], in_=sel[j], compare_op=mybir.AluOpType.is_gt,
                fill=1.0 / GS, base=1 - GS, pattern=[[-GS, GL]],
                channel_multiplier=1,
            )
            nc.gpsimd.affine_select(
                out=sel[j], in_=sel[j], compare_op=mybir.AluOpType.is_ge,
                fill=0.0, base=0, pattern=[[-GS, GL]], channel_multiplier=1,
            )
            nc.gpsimd.memset(selT[j], 0.0)
            nc.gpsimd.affine_select(
                out=selT[j], in_=selT[j], compare_op=mybir.AluOpType.is_gt,
                fill=1.0, base=1 - GS, pattern=[[1, P]], channel_multiplier=-GS,
            )
            nc.gpsimd.affine_select(
                out=selT[j], in_=selT[j], compare_op=mybir.AluOpType.is_ge,
                fill=0.0, base=0, pattern=[[1, P]], channel_multiplier=-GS,
            )

        # q/k bias folded with the attention scale: q' = (q + bq) * SCALE
        bqs_sb = []
        for j in range(CK):
            t = work.tile([P, 1], F32, tag="bqs", name=f"bqs{j}")
            nc.vector.tensor_scalar_mul(out=t, in0=bq_sb[j], scalar1=SCALE)
            bqs_sb.append(t)

        # ---------------- weight transposes (PE) ----------------
        for i in range(O_QKV // P):
            for j in range(CK):
                pt = ps_a.tile([P, P], F32, tag="a", name="wtp")
                nc.tensor.transpose(pt, wq_raw[i][:, j * P:(j + 1) * P], ident_f)
                nc.vector.tensor_copy(wqkvT[j][:, i * P:(i + 1) * P], pt)
        for i in range(CK):
            for j in range(CK):
                pt = ps_a.tile([P, P], F32, tag="a", name="wtp2")
                nc.tensor.transpose(pt, wo_raw[i][:, j * P:(j + 1) * P], ident_f)
                nc.vector.tensor_copy(woutT[j][:, i * P:(i + 1) * P], pt)

        # ---------------- group norm (per-chunk independent) -----------
        for j in range(CK):
            stats = work.tile([P, n // nbsub, 6], F32, tag="bnst", name=f"bnst{j}")
            xr = x_sb[j][:].rearrange("p (s d) -> p s d", d=nbsub)
            for s in range(n // nbsub):
                nc.vector.bn_stats(out=stats[:, s, :], in_=xr[:, s, :])
            mv = work.tile([P, 2], F32, tag="mv", name=f"mv{j}")
            nc.vector.bn_aggr(out=mv, in_=stats)
            # mv2 = [mean, E[x^2]] per channel
            m2 = work.tile([P, 1], F32, tag="m2", name=f"m2{j}")
            nc.vector.tensor_mul(m2, mv[:, 0:1], mv[:, 0:1])
            mv2 = work.tile([P, 2], F32, tag="mv2", name=f"mv2{j}")
            nc.vector.tensor_copy(mv2[:, 0:1], mv[:, 0:1])
            nc.vector.tensor_add(mv2[:, 1:2], mv[:, 1:2], m2)
            ps_g = ps_a.tile([GL, 2], F32, tag="a", name="ps_g")
            nc.tensor.matmul(ps_g, sel[j], mv2, start=True, stop=True)
            gs = work.tile([GL, 2], F32, tag="gs", name=f"gs{j}")
            nc.vector.tensor_copy(gs, ps_g)
            gm2 = work.tile([GL, 1], F32, tag="gm2", name=f"gm2{j}")
            nc.vector.tensor_mul(gm2, gs[:, 0:1], gs[:, 0:1])
            gvar = work.tile([GL, 1], F32, tag="gvar", name=f"gvar{j}")
            nc.vector.tensor_sub(gvar, gs[:, 1:2], gm2)
            gsd = work.tile([GL, 1], F32, tag="gsd", name=f"gsd{j}")
            nc.scalar.activation(out=gsd, in_=gvar,
                                 func=mybir.ActivationFunctionType.Sqrt,
                                 bias=eps_sb, scale=1.0)
            grstd = work.tile([GL, 1], F32, tag="grstd", name=f"grstd{j}")
            nc.vector.reciprocal(grstd, gsd)
            gstat2 = work.tile([GL, 2], F32, tag="gstat2", name=f"gstat2{j}")
            nc.vector.tensor_copy(gstat2[:, 0:1], gs[:, 0:1])
            nc.vector.tensor_copy(gstat2[:, 1:2], grstd)
            ps_bc = ps_a.tile([P, 2], F32, tag="a", name="ps_bc")
            nc.tensor.matmul(ps_bc, selT[j], gstat2, start=True, stop=True)
            a_c = work.tile([P, 1], F32, tag="a_c", name=f"a_c{j}")
            nc.vector.tensor_mul(a_c, ps_bc[:, 1:2], scale_sb[j])
            t_c = work.tile([P, 1], F32, tag="t_c", name=f"t_c{j}")
            nc.vector.tensor_mul(t_c, ps_bc[:, 0:1], a_c)
            b_c = work.tile([P, 1], F32, tag="b_c", name=f"b_c{j}")
            nc.vector.tensor_sub(b_c, bias_sb[j], t_c)
            if j == 0:
                nc.vector.tensor_scalar(
                    out=xn_sb[j], in0=x_sb[j], scalar1=a_c, scalar2=b_c,
                    op0=mybir.AluOpType.mult, op1=mybir.AluOpType.add,
                )
            else:
                # xn = Identity(x * A + B) on ACT frees the DVE
                nc.scalar.activation(
                    out=xn_sb[j], in_=x_sb[j],
                    func=mybir.ActivationFunctionType.Identity,
                    bias=b_c, scale=a_c,
                )

        # preload the exp table set so the first real exp doesn't stall
        # ~1.5us on ACT_TABLE_LOAD (sqrt set was loaded during GroupNorm)
        dummy_exp = work.tile([1, 1], F32, tag="dummy", name="dummy_exp")
        nc.scalar.activation(out=dummy_exp, in_=eps_sb[0:1, :],
                             func=mybir.ActivationFunctionType.Exp)

        # ---------------- q, k projections ----------------
        for nb in range(nblk):
            for idx, (dst, bias_ap, sc) in enumerate(
                [(q3, bqs_sb, SCALE), (k3, bk_sb, 1.0)]
            ):
                for oc in range(CK):
                    o_off = idx * C + oc * P
                    ps = ps_a.tile([P, NB], F32, tag="a", name="ps_qk")
                    for kc in range(CK):
                        nc.tensor.matmul(
                            ps,
                            wqkvT[kc][:, o_off:o_off + P],
                            xn_sb[kc][:, nb * NB:(nb + 1) * NB],
                            start=(kc == 0), stop=(kc == CK - 1),
                        )
                    # DVE-only: ACT Identity evictions here leak into the
                    # attention phase and stall the exp stream (~11us observed)
                    nc.vector.tensor_scalar(
                        out=dst[:, oc, nb * NB:(nb + 1) * NB], in0=ps,
                        scalar1=sc, scalar2=bias_ap[oc],
                        op0=mybir.AluOpType.mult, op1=mybir.AluOpType.add,
                    )

        # ---------------- vT projection (m-part, c-free, +ones col) ------
        nc.gpsimd.memset(vt_sb[:, :, C:C + VPAD], 0.0)
        nc.gpsimd.memset(vt_sb[:, :, C:C + 1], 1.0)
        for mt in range(mtn):
            ps = ps_a.tile([P, NB], F32, tag="a", name="ps_v")
            for kc in range(CK):
                nc.tensor.matmul(
                    ps[:, 0:C],
                    xn_sb[kc][:, mt * P:(mt + 1) * P],
                    wqkvT[kc][:, 2 * C:3 * C],
                    start=(kc == 0), stop=(kc == CK - 1),
                )
            nc.vector.tensor_add(vt_sb[:, mt, 0:C], ps[:, 0:C], bv_bc)

        # ---------------- attention blocks ----------------
        # fp8 DoubleRow: each S matmul contracts K=256 (both channel chunks
        # paired per PE cell); each AO matmul contracts a PAIR of key tiles.
        # The 4 AO accumulation chains trail the exp front inside the
        # S-phase. Each block's normalize result is held and its
        # transpose/project/store tail is emitted early in the NEXT block's
        # S-phase — after the previous chains' psum slots are freed but
        # BEFORE this block's chains claim theirs — so the PE never parks
        # at a block boundary while ACT still has exp backlog.
        expT3 = expT[:]
        DR = mybir.MatmulPerfMode.DoubleRow
        LAG = 4
        npairs = mtn // 2
        nsubs = NB // P

        def emit_s_pair(blk, p):
            nsl = slice(blk * NB, (blk + 1) * NB)
            ps = ps_s.tile([P, 2, NB], F32, tag="s", name="ps_s")
            for sub in range(2):
                mt = 2 * p + sub
                nc.tensor.matmul(
                    ps[:, sub, :],
                    k3[:, :, mt * P:(mt + 1) * P],
                    q3[:, :, nsl],
                    perf_mode=DR, start=True, stop=True,
                )
            nc.scalar.activation(
                out=expT3[:, 2 * p:2 * p + 2, :], in_=ps,
                func=mybir.ActivationFunctionType.Exp,
                bias=eln_sb,
            )

        def emit_ao_pair(ao_ps, j):
            for c in range(nsubs):
                nc.tensor.matmul(
                    ao_ps[c][:, 0:C + VPAD],
                    expT3[:, 2 * j:2 * j + 2, c * P:(c + 1) * P],
                    vt_sb[:, 2 * j:2 * j + 2, :],
                    perf_mode=DR,
                    start=(j == 0), stop=(j == npairs - 1),
                )

        def emit_tail(blk, aots):
            nsl = slice(blk * NB, (blk + 1) * NB)
            ao_sb = [
                evac.tile([P, NB], BF16, tag=f"ao_sb{j}", name=f"ao_sb{j}")
                for j in range(CK)
            ]
            for c in range(nsubs):
                for j in range(CK):
                    pt = ps_a.tile([P, P], BF16, tag="a", name="ao_tp")
                    nc.tensor.transpose(pt, aots[c][:, j * P:(j + 1) * P],
                                        ident_b)
                    nc.vector.tensor_copy(ao_sb[j][:, c * P:(c + 1) * P], pt)
            for oc in range(CK):
                ps = ps_a.tile([P, NB], F32, tag="a", name="ps_f")
                for kc in range(CK):
                    nc.tensor.matmul(
                        ps,
                        woutT[kc][:, oc * P:(oc + 1) * P],
                        ao_sb[kc],
                        start=(kc == 0), stop=(kc == CK - 1),
                    )
                f_sb = evac.tile([P, NB], F32, tag="f_sb", name="f_sb")
                nc.scalar.activation(out=f_sb, in_=ps,
                                     func=mybir.ActivationFunctionType.Identity,
                                     bias=bo_sb[oc], scale=1.0)
                o_sb = evac.tile([P, NB], F32, tag="o_sb", name="o_sb")
                nc.vector.tensor_add(o_sb, f_sb, x_sb[oc][:, nsl])
                nc.sync.dma_start(out=out_d[oc * P:(oc + 1) * P, nsl], in_=o_sb)

        pending = None
        for blk in range(nblk):
            lag = min(LAG, npairs - 1)
            ao_ps = None
            for p in range(npairs):
                emit_s_pair(blk, p)
                if p == min(2, lag - 1) and pending is not None:
                    emit_tail(*pending)
                    pending = None
                if p == lag:
                    ao_ps = [ps_a.tile([P, NB], F32, tag="a", name=f"ps_ao{c}")
                             for c in range(nsubs)]
                if p >= lag:
                    emit_ao_pair(ao_ps, p - lag)
            for j in range(npairs - lag, npairs):
                emit_ao_pair(ao_ps, j)
            aots = []
            for c in range(nsubs):
                ps = ao_ps[c]
                recip = work.tile([P, 1], F32, tag="recip", name="recip")
                nc.vector.reciprocal(recip, ps[:, C:C + 1])
                aot = evac.tile([P, C], BF16, tag="aot", bufs=8, name="aot")
                nc.vector.tensor_scalar_mul(out=aot, in0=ps[:, 0:C], scalar1=recip)
                aots.append(aot)
            pending = (blk, aots)
        emit_tail(*pending)

    return nc


_CACHED_NC = {}


def build_nc(n=N):
    if n not in _CACHED_NC:
        nc = bacc.Bacc("TRN2", target_bir_lowering=False, debug=False,
                       num_devices=B)
        build_attention_program(nc, n=n)
        nc.compile()
        _CACHED_NC[n] = nc
    return _CACHED_NC[n]


def make_in_maps(x, gn_scale, gn_bias, w_qkv, b_qkv, w_out, b_out):
    f = np.ascontiguousarray
    return [
        {
            "x": f(x[b].reshape(C, N), dtype=np.float32),
            "gn_scale": f(gn_scale, dtype=np.float32),
            "gn_bias": f(gn_bias, dtype=np.float32),
            "w_qkv": f(w_qkv, dtype=np.float32),
            "b_qkv": f(b_qkv, dtype=np.float32),
            "w_out": f(w_out, dtype=np.float32),
            "b_out": f(b_out, dtype=np.float32),
        }
        for b in range(B)
    ]


def kernel(x, gn_scale, gn_bias, w_qkv, b_qkv, w_out, b_out, _trace=False,
           _tmpdir=None):
    x = np.asarray(x)
    nc = build_nc()
    in_maps = make_in_maps(x, gn_scale, gn_bias, w_qkv, b_qkv, w_out, b_out)
    res = run_bass_kernel_spmd(nc, in_maps, list(range(B)), trace=_trace,
                               tmpdir=_tmpdir)
    out = np.stack([res.results[b]["out"] for b in range(B)])
    out = out.reshape(B, C, H, W).astype(np.float32)
    if _trace:
        kernel.last_exec_time_ns = res.exec_time_ns
        kernel.last_results = res
    return out
